# revision 19
# baseline (speedup 1.0000x reference)
"""Dense dot-product attention (B=8, S=2048, D=64, fp32) on 8 TRN2 NeuronCores.

Sharding: batch dim across the 8 cores (data parallel), one batch element per
core. Per-core algorithm (fp32 data, matmul dtype configurable):

  Layouts: QT/KT = [D(+1), S] (head-dim on partitions, built via PE transpose),
  V' = [S, D+1] natural (extra ones column).

  The scores matrix is computed TRANSPOSED, ST[k, q] = K @ Q^T; row 64 of
  KT_aug holds (1-mask_k)*8*NEG so the additive key mask rides along as a 65th
  contraction row (exactly 0 for mask=1). exp via ACT (scale=1/8 folds in the
  sqrt(D) scaler). The PV matmul out_T[d, q] = V'^T @ STexp needs no transpose
  of STexp because contraction (k) is already on partitions; V' ones column
  makes row 64 of the accumulator the softmax denominator. A final PE
  transpose returns [q, d(+denom)] natural layout, divide, DMA out.

  All bulk HBM traffic moves as one DMA instruction per tensor (chunk-major
  APs) — many small DMAs serialize on the sync queue and dominated the first
  profile.
"""

import os

import numpy as np

import concourse.bass as bass
import concourse.mybir as mybir
import concourse.tile as tile
from concourse import bacc
from concourse.bass import ts
from concourse.bass_utils import run_bass_kernel_spmd
from concourse.masks import make_identity

B, S, D = 8, 2048, 64
NEG = -1e9
P = 128          # k-chunk height / q-subtile height
NKC = S // P     # 16 k-chunks
EW = 1024        # exp granularity (q width per ST tile)
NE = S // EW     # ST tiles per chunk
MMW = 512        # matmul moving width (one fp32 PSUM bank)
F32 = mybir.dt.float32
F32R = mybir.dt.float32r

_DTMAP = {"f32": F32, "f32r": F32R, "bf16": mybir.dt.bfloat16}
QK_DT = _DTMAP[os.environ.get("QK_DT", "f32r")]
PV_DT = _DTMAP[os.environ.get("PV_DT", "f32r")]

_CACHE: dict = {}


def _build_nc():
    # Bacc (not raw Bass): its compile() splits multi-wait sync lists into
    # event semaphores — TRN2 instructions carry at most ONE sync wait.
    nc = bacc.Bacc("TRN2", target_bir_lowering=False, debug=False)

    q = nc.dram_tensor("q", [S, D], F32, kind="ExternalInput").ap()
    k = nc.dram_tensor("k", [S, D], F32, kind="ExternalInput").ap()
    v = nc.dram_tensor("v", [S, D], F32, kind="ExternalInput").ap()
    mk = nc.dram_tensor("mk", [S], F32, kind="ExternalInput").ap()
    mv = nc.dram_tensor("mv", [S], F32, kind="ExternalInput").ap()
    out = nc.dram_tensor("out", [S, D], F32, kind="ExternalOutput").ap()

    with tile.TileContext(nc) as tc:
        with (
            tc.tile_pool(name="const", bufs=1) as const,
            tc.tile_pool(name="stexp", bufs=4) as stexp_pool,
            tc.tile_pool(name="pvsb", bufs=3) as pvsb_pool,
        ):
            ident = const.tile([P, P], F32)
            make_identity(nc, ident)

            qt = const.tile([D + 1, S], QK_DT, tag="qt")
            kt = const.tile([D + 1, S], QK_DT, tag="kt")
            vp = const.tile([P, NKC, D + 1], PV_DT, tag="vp")
            mk_sb = const.tile([1, S], F32, tag="mk")
            mv_sb = const.tile([P, NKC], F32, tag="mv")
            qf = const.tile([P, NKC, D], F32, tag="qf")
            kf = const.tile([P, NKC, D], F32, tag="kf")
            vf = const.tile([P, NKC, D], F32, tag="vf")
            ob = const.tile([P, NKC, D], F32, tag="ob")

            # DMA order follows the main loop's critical path: masks first
            # (row-64 writes gate every ST matmul), then q/k interleaved so
            # the first chunks of BOTH arrive early, V last.
            nc.sync.dma_start(out=mk_sb, in_=mk.rearrange("(a s) -> a s", a=1))
            nc.sync.dma_start(out=mv_sb, in_=mv.rearrange("(n p) -> p n", p=P))
            qr = q.rearrange("(n p) d -> p n d", p=P)
            kr = k.rearrange("(n p) d -> p n d", p=P)
            vr = v.rearrange("(n p) d -> p n d", p=P)
            DSP = 4  # DMA split: overlap transposes with load
            for i in range(DSP):
                sl = slice(i * NKC // DSP, (i + 1) * NKC // DSP)
                nc.sync.dma_start(out=qf[:, sl, :], in_=qr[:, sl, :])
                nc.sync.dma_start(out=kf[:, sl, :], in_=kr[:, sl, :])
            for i in range(DSP):
                sl = slice(i * NKC // DSP, (i + 1) * NKC // DSP)
                nc.sync.dma_start(out=vf[:, sl, :], in_=vr[:, sl, :])

            # augmentation rows: QT row 64 = 1, KT row 64 = (1-mask_k)*8*NEG
            # — written before anything else so no ST matmul waits on them
            ones_row = const.tile([1, S], F32, tag="ones_row")
            nc.vector.memset(ones_row, 1.0)
            nc.vector.tensor_copy(qt[D : D + 1, :], ones_row)
            nc.vector.tensor_scalar(
                kt[D : D + 1, :],
                mk_sb,
                -8.0 * NEG,
                8.0 * NEG,
                op0=mybir.AluOpType.mult,
                op1=mybir.AluOpType.add,
            )

            # Q^T / K^T via PE transpose of natural [128, 64] chunks.
            # Emission order = ST(0) dependency order: q0..q3, k0, q4..q7,
            # k1, then the rest.
            tp_order = (
                [("q", i) for i in range(4)]
                + [("k", 0)]
                + [("q", i) for i in range(4, 8)]
                + [("k", 1)]
            )
            for n in range(2, NKC):
                tp_order.append(("k", n))
                if 8 + (n - 2) < NKC:
                    tp_order.append(("q", 8 + (n - 2)))
            with tc.tile_pool(name="tp_ps", bufs=4, space="PSUM") as tp_ps:
                for which, n in tp_order:
                    tp = tp_ps.tile([D, P], F32, tag="tps")
                    if which == "q":
                        nc.tensor.transpose(tp, qf[:, n, :], ident)
                        nc.vector.tensor_copy(qt[0:D, ts(n, P)], tp)
                    else:
                        nc.tensor.transpose(tp, kf[:, n, :], ident)
                        nc.vector.tensor_copy(kt[0:D, ts(n, P)], tp)

            # V' chunks: [128, 65] with cols 0:64 = V*mask_v, col 64 = 1.0
            # (memset cannot write f32r, so ones go through an f32 staging
            # tile and a converting copy)
            ones_col = const.tile([P, NKC, 1], F32, tag="ones_col")
            nc.vector.memset(ones_col, 1.0)
            nc.vector.tensor_copy(vp[:, :, D : D + 1], ones_col)
            nc.vector.tensor_tensor(
                vp[:, :, 0:D],
                vf,
                mv_sb[:, :, None].to_broadcast([P, NKC, D]),
                mybir.AluOpType.mult,
            )

            # Main loop: e (q-stripe) outer, then the epilogue for each
            # stripe right after its last PV — Tile overlaps what deps allow.
            with tc.tile_pool(name="pv_ps", bufs=1, space="PSUM") as pv_ps_pool:
                pv = pv_ps_pool.tile([D + 1, S], F32, tag="pv")
                with tc.tile_pool(name="st_ps", bufs=2, space="PSUM") as st_ps:
                    for e in range(NE):
                        for n in range(NKC):
                            st = st_ps.tile([P, EW], F32, tag="st")
                            for h in range(EW // MMW):
                                nc.tensor.matmul(
                                    st[:, ts(h, MMW)],
                                    lhsT=kt[:, ts(n, P)],
                                    rhs=qt[:, ts(e * (EW // MMW) + h, MMW)],
                                    start=True,
                                    stop=True,
                                )
                            se = stexp_pool.tile([P, EW], PV_DT, tag="se")
                            nc.scalar.activation(
                                se, st, mybir.ActivationFunctionType.Exp, scale=0.125
                            )
                            for h in range(EW // MMW):
                                nc.tensor.matmul(
                                    pv[:, ts(e * (EW // MMW) + h, MMW)],
                                    lhsT=vp[:, n, :],
                                    rhs=se[:, ts(h, MMW)],
                                    start=(n == 0),
                                    stop=(n == NKC - 1),
                                )
                # Epilogue: transpose back to [q, d], divide by the
                # denominator row, stage into ob, single output DMA
                with tc.tile_pool(name="ep_ps", bufs=3, space="PSUM") as ep_ps:
                    for m in range(NKC):
                        pvsb = pvsb_pool.tile([D + 1, P], F32, tag="pvsb")
                        nc.vector.tensor_copy(pvsb, pv[:, ts(m, P)])
                        ot = ep_ps.tile([P, D + 1], F32, tag="ot")
                        nc.tensor.transpose(ot, pvsb, ident[0 : D + 1, 0 : D + 1])
                        rec = pvsb_pool.tile([P, 1], F32, tag="rec")
                        nc.vector.reciprocal(rec, ot[:, D : D + 1])
                        nc.vector.tensor_tensor(
                            ob[:, m, :],
                            ot[:, 0:D],
                            rec.to_broadcast([P, D]),
                            mybir.AluOpType.mult,
                        )
                    nc.sync.dma_start(
                        out=out.rearrange("(n p) d -> p n d", p=P), in_=ob
                    )

    nc.compile()
    return nc


def get_nc():
    if "nc" not in _CACHE:
        _CACHE["nc"] = _build_nc()
    return _CACHE["nc"]


def kernel(queries, keys, values, mask_q, mask_k, mask_v, **_unused):
    nc = get_nc()
    in_maps = [
        {
            "q": np.ascontiguousarray(queries[b], dtype=np.float32),
            "k": np.ascontiguousarray(keys[b], dtype=np.float32),
            "v": np.ascontiguousarray(values[b], dtype=np.float32),
            "mk": np.ascontiguousarray(mask_k[b], dtype=np.float32),
            "mv": np.ascontiguousarray(mask_v[b], dtype=np.float32),
        }
        for b in range(B)
    ]
    res = run_bass_kernel_spmd(nc, in_maps, core_ids=list(range(B)))
    return np.stack([res.results[b]["out"] for b in range(B)], axis=0)
